# revision 3
# baseline (speedup 1.0000x reference)
"""Mamba chunk-state kernel for Trainium2 (8 NeuronCores, Bass/Tile).

states[b,c,h,p,n] = sum_l x[b,c,l,h,p] * scale[b,h,c,l] * B[b,c,l,n]
scale = exp(dA[...,-1:] - dA) * dt

Memory-roofline design (the 8 cores share one chip's HBM, ~340 GB/s/core
effective; per-core traffic is 27.5 MB -> ~80 us floor):
  - fp16 for x / B / dt / dA / output (tolerance 2e-2; measured rel err
    ~1.3e-3): halves HBM traffic vs f32, runs PE at 1 cycle/row (fp32
    is 4), and beats bf16 by 4 mantissa bits (all values fit fp16 range;
    max |xw| ~4e3 << 65504).
  - x uploaded p-major ([s, p*32+h]) so the per-(h,l) scale multiply is a
    SINGLE DVE tensor_tensor per l-half with the small scale tile read
    through a stride-0 broadcast AP (innermost h stays packed -> 2x DVE
    perf mode), replacing 64 per-head tensor_scalar ops (~100 us DVE).
  - matmuls flipped: lhsT = B (m=128 dstate on PSUM partitions), rhs = xw
    (n=512 moving cols, one 4-bank PSUM tile per chunk) -> 8 bf16 matmuls
    per chunk; output leaves transposed [n, (p,h)] and the host
    untransposes during assembly.
  - scale computed in [l, h] layout directly (dt/dA uploaded as [s, h],
    dA_last replicated per chunk on host) -> no PE transpose, PSUM holds
    exactly two 4-bank chunk tiles (full double buffering).
  - every input is pre-arranged on host into its exact SBUF partition
    image and packed by dtype, so each 4-chunk superstep is 3 DMAs
    (x+B bf16 load, dt/dA/dal f32 load, store), all with >=4 KB
    contiguous descriptors (no <512B read-modify-write penalty).

Sharding: core i handles batch b = i//2 and chunk range (i%2)*16..+16.
Fully independent slices -> no collectives.
"""

import numpy as np
import ml_dtypes

BF16 = np.float16

BATCH, SEQLEN, NGROUPS, DSTATE = 4, 8192, 1, 128
NHEADS, HEADDIM, CHUNK = 32, 64, 256
NCHUNKS = SEQLEN // CHUNK  # 32
NCORES = 8
CPC = (BATCH * NCHUNKS) // NCORES  # 16 chunks per core
HP = NHEADS * HEADDIM  # 2048
R = CPC * CHUNK  # 4096 rows per core
GROUP = 4  # chunks per superstep
NG = CPC // GROUP  # 4 supersteps

_cached_nc = None


def _build_nc(repeat=1, loop_trips=None, body_passes=4):
    import concourse.bacc as bacc
    import concourse.mybir as mybir
    import concourse.tile as tile

    f32 = mybir.dt.float32
    bf16 = mybir.dt.float16  # fp16: same width, 4 more mantissa bits
    Exp = mybir.ActivationFunctionType.Exp

    nc = bacc.Bacc(
        "TRN2",
        target_bir_lowering=False,
        debug=False,
        num_devices=NCORES,
    )

    G2 = GROUP * 2  # (chunk, half) blocks per superstep
    WXB = G2 * (HP + DSTATE)  # bf16 pack: x blocks then B blocks
    WMETA = G2 * NHEADS * 2 + GROUP * NHEADS  # f32 pack: dt, dA, dal
    xb_d = nc.dram_tensor("xb_s", [NG, 128, WXB], bf16, kind="ExternalInput").ap()
    mt_d = nc.dram_tensor("mt_s", [NG, 128, WMETA], bf16, kind="ExternalInput").ap()
    out_d = nc.dram_tensor(
        "out_s", [NG, DSTATE, GROUP * HP], f32 if False else bf16, kind="ExternalOutput"
    ).ap()

    with tile.TileContext(nc) as tc:
        with (
            tc.tile_pool(name="xin", bufs=2) as x_pool,
            tc.tile_pool(name="bin", bufs=2) as b_pool,
            tc.tile_pool(name="meta", bufs=2) as m_pool,
            tc.tile_pool(name="scp", bufs=6) as sc_pool,
            tc.tile_pool(name="xwp", bufs=3) as xw_pool,
            tc.tile_pool(name="stgp", bufs=2) as stg_pool,
            tc.tile_pool(name="pstates", bufs=2, space="PSUM") as ps_pool,
        ):
            import contextlib

            if loop_trips is not None:
                loop_cm = tc.For_i(0, loop_trips)
                n_body = NG * body_passes
            else:
                loop_cm = contextlib.nullcontext()
                n_body = NG * repeat
            with loop_cm:
              for g_rep in range(n_body):
                g = g_rep % NG
                r0 = g * GROUP * CHUNK  # row offset in R-space
                # ---- superstep loads (one DMA per tensor) ----
                xbg = x_pool.tile([128, WXB], bf16, name="xbg", tag="xbg")
                nc.sync.dma_start(xbg[:], xb_d[g])
                mtg = m_pool.tile([128, WMETA], bf16, name="mtg", tag="mtg")
                nc.sync.dma_start(mtg[:], mt_d[g])
                xg = xbg[:, : G2 * HP]
                bg = xbg[:, G2 * HP :]
                dtg = mtg[:, : G2 * NHEADS]
                dag = mtg[:, G2 * NHEADS : 2 * G2 * NHEADS]
                dalg = mtg[:, 2 * G2 * NHEADS :]

                stg = stg_pool.tile([128, GROUP * HP], bf16, name="stg", tag="stg")

                # ---- scale = exp(dA_last - dA) * dt for all chunks first,
                # so the tiny ACT exps don't queue behind big evict copies ----
                scts = []
                for k in range(GROUP):
                    diff = sc_pool.tile([128, 2 * NHEADS], f32, name="dif", tag="dif")
                    nc.gpsimd.tensor_sub(
                        diff.rearrange("l (f h) -> l f h", f=2),
                        dalg[:, k * NHEADS : (k + 1) * NHEADS]
                        .unsqueeze(1)
                        .broadcast_to((128, 2, NHEADS)),
                        dag[:, k * 2 * NHEADS : (k + 1) * 2 * NHEADS].rearrange(
                            "l (f h) -> l f h", f=2
                        ),
                    )
                    e = sc_pool.tile([128, 2 * NHEADS], f32, name="e", tag="e")
                    nc.scalar.activation(e[:], diff[:], Exp)
                    sct = sc_pool.tile([128, 2 * NHEADS], bf16, name="sct", tag="sct")
                    nc.gpsimd.tensor_mul(
                        sct[:], e[:], dtg[:, k * 2 * NHEADS : (k + 1) * 2 * NHEADS]
                    )
                    scts.append(sct)

                for k in range(GROUP):
                    # ---- xw = x * scale: ONE DVE op via broadcast AP ----
                    xw = xw_pool.tile([128, 2 * HP], bf16, name="xw", tag="xw")
                    nc.vector.tensor_mul(
                        xw.rearrange("l (f p h) -> l f p h", f=2, p=HEADDIM),
                        xg[:, k * 2 * HP : (k + 1) * 2 * HP].rearrange(
                            "l (f p h) -> l f p h", f=2, p=HEADDIM
                        ),
                        scts[k]
                        .rearrange("l (f h) -> l f h", f=2)
                        .unsqueeze(2)
                        .broadcast_to((128, 2, HEADDIM, NHEADS)),
                    )

                    # ---- states^T: lhsT=B (m=dstate), rhs=xw (n=512) ----
                    st = ps_pool.tile([128, HP], f32, name="st", tag="st")
                    for f in range(2):
                        for q in range(4):
                            nc.tensor.matmul(
                                st[:, q * 512 : (q + 1) * 512],
                                bg[:, (k * 2 + f) * DSTATE : (k * 2 + f + 1) * DSTATE],
                                xw[:, f * HP + q * 512 : f * HP + (q + 1) * 512],
                                start=(f == 0),
                                stop=(f == 1),
                            )
                    nc.scalar.copy(stg[:, k * HP : (k + 1) * HP], st[:])

                # ---- one store DMA per superstep ----
                nc.scalar.dma_start(out_d[g], stg[:])

    nc.compile()
    return nc


def _get_nc():
    global _cached_nc
    if _cached_nc is None:
        _cached_nc = _build_nc()
    return _cached_nc


def _in_maps(B, x, dt, dA_cumsum):
    B = np.asarray(B, dtype=np.float32)
    x = np.asarray(x, dtype=np.float32)
    dt = np.asarray(dt, dtype=np.float32)
    dA = np.asarray(dA_cumsum, dtype=np.float32)
    maps = []
    for core in range(NCORES):
        b = core // 2
        c0 = (core % 2) * CPC
        s0, s1 = c0 * CHUNK, (c0 + CPC) * CHUNK
        # x -> p-major [s, p*32+h], bf16
        xs = (
            np.ascontiguousarray(x[b, s0:s1].transpose(0, 2, 1))
            .reshape(R, HP)
            .astype(BF16)
        )
        bs = np.ascontiguousarray(B[b, s0:s1, 0, :]).astype(BF16)
        # dt/dA -> [s, h] f32
        dts = np.ascontiguousarray(
            dt[b, :, c0 : c0 + CPC, :].transpose(1, 2, 0)
        ).reshape(R, NHEADS)
        das = np.ascontiguousarray(
            dA[b, :, c0 : c0 + CPC, :].transpose(1, 2, 0)
        ).reshape(R, NHEADS)
        # dA_last replicated to [c*128+l, h] f32
        dal = np.repeat(
            np.ascontiguousarray(dA[b, :, c0 : c0 + CPC, -1].T), 128, axis=0
        )
        def pimg(arr, blocks, w):
            # [NG*blocks*128, w] -> [NG, 128, blocks*w] partition image
            return np.ascontiguousarray(
                arr.reshape(NG, blocks, 128, w).transpose(0, 2, 1, 3)
            ).reshape(NG, 128, blocks * w)

        maps.append(
            {
                "xb_s": np.concatenate(
                    [pimg(xs, GROUP * 2, HP), pimg(bs, GROUP * 2, DSTATE)], axis=2
                ),
                "mt_s": np.concatenate(
                    [
                        pimg(dts, GROUP * 2, NHEADS),
                        pimg(das, GROUP * 2, NHEADS),
                        pimg(dal, GROUP, NHEADS),
                    ],
                    axis=2,
                ).astype(BF16),
            }
        )
    return maps


def _assemble(results):
    out = np.empty((BATCH, NCHUNKS, NHEADS, HEADDIM, DSTATE), np.float32)
    for core in range(NCORES):
        b = core // 2
        c0 = (core % 2) * CPC
        o = np.asarray(results[core]["out_s"]).astype(np.float32)
        # [NG, n, k*2048 + p*32 + h] -> [c, h, p, n]
        o = o.reshape(NG, DSTATE, GROUP, HEADDIM, NHEADS)
        out[b, c0 : c0 + CPC] = o.transpose(0, 2, 4, 3, 1).reshape(
            CPC, NHEADS, HEADDIM, DSTATE
        )
    return out


def _run(B, x, dt, dA_cumsum, **run_kwargs):
    from concourse import bass_utils

    nc = _get_nc()
    res = bass_utils.run_bass_kernel_spmd(
        nc, _in_maps(B, x, dt, dA_cumsum), core_ids=list(range(NCORES)), **run_kwargs
    )
    return _assemble(res.results), res


def kernel(B, x, dt, dA_cumsum):
    out, _ = _run(B, x, dt, dA_cumsum)
    return out


# revision 4
# speedup vs baseline: 1.0779x; 1.0779x over previous
"""Mamba chunk-state kernel for Trainium2 (8 NeuronCores, Bass/Tile).

states[b,c,h,p,n] = sum_l x[b,c,l,h,p] * scale[b,h,c,l] * B[b,c,l,n]
scale = exp(dA[...,-1:] - dA) * dt

Memory-roofline design (the 8 cores share one chip's HBM, ~340 GB/s/core
effective; per-core traffic is 27.5 MB -> ~80 us floor):
  - fp16 for x / B / dt / dA / output (tolerance 2e-2; measured rel err
    ~1.3e-3): halves HBM traffic vs f32, runs PE at 1 cycle/row (fp32
    is 4), and beats bf16 by 4 mantissa bits (all values fit fp16 range;
    max |xw| ~4e3 << 65504).
  - x uploaded p-major ([s, p*32+h]) so the per-(h,l) scale multiply is a
    SINGLE DVE tensor_tensor per l-half with the small scale tile read
    through a stride-0 broadcast AP (innermost h stays packed -> 2x DVE
    perf mode), replacing 64 per-head tensor_scalar ops (~100 us DVE).
  - matmuls flipped: lhsT = B (m=128 dstate on PSUM partitions), rhs = xw
    (n=512 moving cols, one 4-bank PSUM tile per chunk) -> 8 bf16 matmuls
    per chunk; output leaves transposed [n, (p,h)] and the host
    untransposes during assembly.
  - scale computed in [l, h] layout directly (dt/dA uploaded as [s, h],
    dA_last replicated per chunk on host) -> no PE transpose, PSUM holds
    exactly two 4-bank chunk tiles (full double buffering).
  - every input is pre-arranged on host into its exact SBUF partition
    image and packed by dtype, so each 4-chunk superstep is 3 DMAs
    (x+B bf16 load, dt/dA/dal f32 load, store), all with >=4 KB
    contiguous descriptors (no <512B read-modify-write penalty).

Sharding: core i handles batch b = i//2 and chunk range (i%2)*16..+16.
Fully independent slices -> no collectives.
"""

import numpy as np
import ml_dtypes

BF16 = np.float16

BATCH, SEQLEN, NGROUPS, DSTATE = 4, 8192, 1, 128
NHEADS, HEADDIM, CHUNK = 32, 64, 256
NCHUNKS = SEQLEN // CHUNK  # 32
NCORES = 8
CPC = (BATCH * NCHUNKS) // NCORES  # 16 chunks per core
HP = NHEADS * HEADDIM  # 2048
R = CPC * CHUNK  # 4096 rows per core
GROUP = 4  # chunks per superstep
NG = CPC // GROUP  # 4 supersteps

_cached_nc = None


def _build_nc(repeat=1, loop_trips=None, body_passes=4):
    import concourse.bacc as bacc
    import concourse.mybir as mybir
    import concourse.tile as tile

    f32 = mybir.dt.float32
    bf16 = mybir.dt.float16  # fp16: same width, 4 more mantissa bits
    Exp = mybir.ActivationFunctionType.Exp

    nc = bacc.Bacc(
        "TRN2",
        target_bir_lowering=False,
        debug=False,
        num_devices=NCORES,
    )

    G2 = GROUP * 2  # (chunk, half) blocks per superstep
    WXB = G2 * (HP + DSTATE)  # bf16 pack: x blocks then B blocks
    WMETA = G2 * NHEADS * 2 + GROUP * NHEADS  # f32 pack: dt, dA, dal
    xb_d = nc.dram_tensor("xb_s", [NG, 128, WXB], bf16, kind="ExternalInput").ap()
    mt_d = nc.dram_tensor("mt_s", [NG, 128, WMETA], bf16, kind="ExternalInput").ap()
    out_d = nc.dram_tensor(
        "out_s", [NG, DSTATE, GROUP * HP], f32 if False else bf16, kind="ExternalOutput"
    ).ap()

    with tile.TileContext(nc) as tc:
        with (
            tc.tile_pool(name="xin", bufs=3) as x_pool,
            tc.tile_pool(name="bin", bufs=2) as b_pool,
            tc.tile_pool(name="meta", bufs=2) as m_pool,
            tc.tile_pool(name="scp", bufs=6) as sc_pool,
            tc.tile_pool(name="xwp", bufs=3) as xw_pool,
            tc.tile_pool(name="stgp", bufs=3) as stg_pool,
            tc.tile_pool(name="pstates", bufs=2, space="PSUM") as ps_pool,
        ):
            import contextlib

            if loop_trips is not None:
                loop_cm = tc.For_i(0, loop_trips)
                n_body = NG * body_passes
            else:
                loop_cm = contextlib.nullcontext()
                n_body = NG * repeat
            with loop_cm:
              for g_rep in range(n_body):
                g = g_rep % NG
                r0 = g * GROUP * CHUNK  # row offset in R-space
                # ---- superstep loads (one DMA per tensor) ----
                xbg = x_pool.tile([128, WXB], bf16, name="xbg", tag="xbg")
                nc.sync.dma_start(xbg[:], xb_d[g])
                mtg = m_pool.tile([128, WMETA], bf16, name="mtg", tag="mtg")
                nc.sync.dma_start(mtg[:], mt_d[g])
                xg = xbg[:, : G2 * HP]
                bg = xbg[:, G2 * HP :]
                dtg = mtg[:, : G2 * NHEADS]
                dag = mtg[:, G2 * NHEADS : 2 * G2 * NHEADS]
                dalg = mtg[:, 2 * G2 * NHEADS :]

                stg = stg_pool.tile([128, GROUP * HP], bf16, name="stg", tag="stg")

                # ---- scale = exp(dA_last - dA) * dt for all chunks first,
                # so the tiny ACT exps don't queue behind big evict copies ----
                scts = []
                for k in range(GROUP):
                    diff = sc_pool.tile([128, 2 * NHEADS], f32, name="dif", tag="dif")
                    nc.gpsimd.tensor_sub(
                        diff.rearrange("l (f h) -> l f h", f=2),
                        dalg[:, k * NHEADS : (k + 1) * NHEADS]
                        .unsqueeze(1)
                        .broadcast_to((128, 2, NHEADS)),
                        dag[:, k * 2 * NHEADS : (k + 1) * 2 * NHEADS].rearrange(
                            "l (f h) -> l f h", f=2
                        ),
                    )
                    e = sc_pool.tile([128, 2 * NHEADS], f32, name="e", tag="e")
                    nc.scalar.activation(e[:], diff[:], Exp)
                    sct = sc_pool.tile([128, 2 * NHEADS], bf16, name="sct", tag="sct")
                    nc.gpsimd.tensor_mul(
                        sct[:], e[:], dtg[:, k * 2 * NHEADS : (k + 1) * 2 * NHEADS]
                    )
                    scts.append(sct)

                for k in range(GROUP):
                    # ---- xw = x * scale: ONE DVE op via broadcast AP ----
                    xw = xw_pool.tile([128, 2 * HP], bf16, name="xw", tag="xw")
                    nc.vector.tensor_mul(
                        xw.rearrange("l (f p h) -> l f p h", f=2, p=HEADDIM),
                        xg[:, k * 2 * HP : (k + 1) * 2 * HP].rearrange(
                            "l (f p h) -> l f p h", f=2, p=HEADDIM
                        ),
                        scts[k]
                        .rearrange("l (f h) -> l f h", f=2)
                        .unsqueeze(2)
                        .broadcast_to((128, 2, HEADDIM, NHEADS)),
                    )

                    # ---- states^T: lhsT=B (m=dstate), rhs=xw (n=512) ----
                    st = ps_pool.tile([128, HP], f32, name="st", tag="st")
                    for f in range(2):
                        for q in range(4):
                            nc.tensor.matmul(
                                st[:, q * 512 : (q + 1) * 512],
                                bg[:, (k * 2 + f) * DSTATE : (k * 2 + f + 1) * DSTATE],
                                xw[:, f * HP + q * 512 : f * HP + (q + 1) * 512],
                                start=(f == 0),
                                stop=(f == 1),
                            )
                    nc.scalar.copy(stg[:, k * HP : (k + 1) * HP], st[:])

                # ---- one store DMA per superstep ----
                nc.scalar.dma_start(out_d[g], stg[:])

    nc.compile()
    return nc


def _get_nc():
    global _cached_nc
    if _cached_nc is None:
        _cached_nc = _build_nc()
    return _cached_nc


def _in_maps(B, x, dt, dA_cumsum):
    B = np.asarray(B, dtype=np.float32)
    x = np.asarray(x, dtype=np.float32)
    dt = np.asarray(dt, dtype=np.float32)
    dA = np.asarray(dA_cumsum, dtype=np.float32)
    maps = []
    for core in range(NCORES):
        b = core // 2
        c0 = (core % 2) * CPC
        s0, s1 = c0 * CHUNK, (c0 + CPC) * CHUNK
        # x -> p-major [s, p*32+h], bf16
        xs = (
            np.ascontiguousarray(x[b, s0:s1].transpose(0, 2, 1))
            .reshape(R, HP)
            .astype(BF16)
        )
        bs = np.ascontiguousarray(B[b, s0:s1, 0, :]).astype(BF16)
        # dt/dA -> [s, h] f32
        dts = np.ascontiguousarray(
            dt[b, :, c0 : c0 + CPC, :].transpose(1, 2, 0)
        ).reshape(R, NHEADS)
        das = np.ascontiguousarray(
            dA[b, :, c0 : c0 + CPC, :].transpose(1, 2, 0)
        ).reshape(R, NHEADS)
        # dA_last replicated to [c*128+l, h] f32
        dal = np.repeat(
            np.ascontiguousarray(dA[b, :, c0 : c0 + CPC, -1].T), 128, axis=0
        )
        def pimg(arr, blocks, w):
            # [NG*blocks*128, w] -> [NG, 128, blocks*w] partition image
            return np.ascontiguousarray(
                arr.reshape(NG, blocks, 128, w).transpose(0, 2, 1, 3)
            ).reshape(NG, 128, blocks * w)

        maps.append(
            {
                "xb_s": np.concatenate(
                    [pimg(xs, GROUP * 2, HP), pimg(bs, GROUP * 2, DSTATE)], axis=2
                ),
                "mt_s": np.concatenate(
                    [
                        pimg(dts, GROUP * 2, NHEADS),
                        pimg(das, GROUP * 2, NHEADS),
                        pimg(dal, GROUP, NHEADS),
                    ],
                    axis=2,
                ).astype(BF16),
            }
        )
    return maps


def _assemble(results):
    out = np.empty((BATCH, NCHUNKS, NHEADS, HEADDIM, DSTATE), np.float32)
    for core in range(NCORES):
        b = core // 2
        c0 = (core % 2) * CPC
        o = np.asarray(results[core]["out_s"]).astype(np.float32)
        # [NG, n, k*2048 + p*32 + h] -> [c, h, p, n]
        o = o.reshape(NG, DSTATE, GROUP, HEADDIM, NHEADS)
        out[b, c0 : c0 + CPC] = o.transpose(0, 2, 4, 3, 1).reshape(
            CPC, NHEADS, HEADDIM, DSTATE
        )
    return out


def _run(B, x, dt, dA_cumsum, **run_kwargs):
    from concourse import bass_utils

    nc = _get_nc()
    res = bass_utils.run_bass_kernel_spmd(
        nc, _in_maps(B, x, dt, dA_cumsum), core_ids=list(range(NCORES)), **run_kwargs
    )
    return _assemble(res.results), res


def kernel(B, x, dt, dA_cumsum):
    out, _ = _run(B, x, dt, dA_cumsum)
    return out
